# revision 9
# baseline (speedup 1.0000x reference)
import sys

for _p in ("/opt/trn_rl_repo", "/root/.axon_site/_ro/trn_rl_repo"):
    if _p not in sys.path:
        sys.path.insert(0, _p)

import numpy as np
import jax
import jax.numpy as jnp

from concourse import bass, mybir, tile
from concourse import bass_utils, bacc
from concourse.masks import make_identity

F32 = mybir.dt.float32
I16 = mybir.dt.int16
RELU = mybir.ActivationFunctionType.Relu
OP_MAX = mybir.AluOpType.max
OP_ADD = mybir.AluOpType.add
OP_MULT = mybir.AluOpType.mult
AX_X = mybir.AxisListType.X

TRACE = False
LAST_RESULTS = None
LAST_EXEC_S = None

# logical weight matrices, stored transposed (cin, cout), blocked 128x128
WSPEC = {
    "sa1l1": (3, 64), "sa1l2": (64, 64), "sa1l3": (64, 128),
    "sa2l1a": (3, 128), "sa2l1b": (128, 128), "sa2l2": (128, 128), "sa2l3": (128, 256),
    "sa3l1a": (3, 256), "sa3l1b": (256, 256), "sa3l2": (256, 256), "sa3l3": (256, 512),
    "fp3za": (512, 256), "fp3l1a": (256, 256), "fp3l2": (256, 256),
    "fp2za": (256, 256), "fp2l1a": (128, 256), "fp2l2": (256, 128),
    "fp1z": (128, 128), "fp1l2": (128, 128), "fp1l3": (128, 128),
    "cls1": (128, 128), "cls2": (128, 2),
}


def _nblk(c):
    return (c + 127) // 128


# ----------------------------------------------------------------- host (jax cpu)

def _sqdist(a, b):
    return (jnp.sum(a * a, -1)[:, :, None] + jnp.sum(b * b, -1)[:, None, :]
            - 2.0 * jnp.einsum('bmc,bpc->bmp', a, b))


def _fps(xyz, npoint):
    Bn, Nn, _ = xyz.shape

    def step(carry, _):
        dist, far = carry
        centroid = jnp.take_along_axis(xyz, far[:, None, None], axis=1)
        d = jnp.sum((xyz - centroid) ** 2, -1)
        dist = jnp.minimum(dist, d)
        return (dist, jnp.argmax(dist, axis=1)), far

    init = (jnp.full((Bn, Nn), 1e10, jnp.float32), jnp.zeros((Bn,), jnp.int32))
    _, idx = jax.lax.scan(step, init, None, length=npoint)
    return jnp.transpose(idx)


_gather = jax.vmap(lambda p, i: p[i])


def _host_fn(xyz):
    l0 = jnp.transpose(xyz, (0, 2, 1))
    fi1 = _fps(l0, 2048)
    new1 = _gather(l0, fi1)
    idx1 = jax.lax.top_k(-_sqdist(new1, l0), 32)[1]
    gx1 = _gather(l0, idx1) - new1[:, :, None]
    fi2 = _fps(new1, 512)
    new2 = _gather(new1, fi2)
    idx2 = jax.lax.top_k(-_sqdist(new2, new1), 32)[1]
    gx2 = _gather(new1, idx2) - new2[:, :, None]
    fi3 = _fps(new2, 128)
    new3 = _gather(new2, fi3)
    idx3 = jax.lax.top_k(-_sqdist(new3, new2), 32)[1]
    gx3 = _gather(new2, idx3) - new3[:, :, None]

    def fpw(x1, x2):
        negd, idx = jax.lax.top_k(-_sqdist(x1, x2), 3)
        d = jnp.maximum(-negd, 1e-10)
        w = 1.0 / d
        w = w / jnp.sum(w, -1, keepdims=True)
        return idx, w

    i3f, w3 = fpw(new2, new3)
    i2f, w2 = fpw(new1, new2)
    i1f, w1 = fpw(l0, new1)
    return dict(gx1=gx1, gx2=gx2, gx3=gx3, idx2=idx2, idx3=idx3,
                i3f=i3f, w3=w3, i2f=i2f, w2=w2, i1f=i1f, w1=w1)


_HOST_JIT = None


def _host_indices(xyz):
    global _HOST_JIT
    cpu = jax.devices("cpu")[0]
    with jax.default_device(cpu):
        if _HOST_JIT is None:
            _HOST_JIT = jax.jit(_host_fn)
        res = _HOST_JIT(jax.device_put(np.asarray(xyz, np.float32), cpu))
        return {k: np.asarray(v) for k, v in res.items()}


def _prep_weights(params):
    def npa(x):
        return np.asarray(x, dtype=np.float32)

    out = {}

    def blocks(tag, W, g):
        Wt = (npa(W) * npa(g)[:, None]).T  # (cin, cout)
        cin, cout = Wt.shape
        assert (cin, cout) == WSPEC[tag], (tag, Wt.shape)
        for ki in range(0, cin, 128):
            for mi in range(0, cout, 128):
                out[f"{tag}_{ki // 128}_{mi // 128}"] = \
                    np.ascontiguousarray(Wt[ki:ki + 128, mi:mi + 128])

    sa1, sa2, sa3 = params["sa1"], params["sa2"], params["sa3"]
    fp3, fp2, fp1, cls1 = params["fp3"], params["fp2"], params["fp1"], params["cls1"]

    blocks("sa1l1", npa(sa1[0][0]), sa1[0][1])
    blocks("sa1l2", npa(sa1[1][0]), sa1[1][1])
    blocks("sa1l3", npa(sa1[2][0]), sa1[2][1])

    W, g = npa(sa2[0][0]), sa2[0][1]
    blocks("sa2l1a", W[:, :3], g)
    blocks("sa2l1b", W[:, 3:], g)
    blocks("sa2l2", npa(sa2[1][0]), sa2[1][1])
    blocks("sa2l3", npa(sa2[2][0]), sa2[2][1])

    W, g = npa(sa3[0][0]), sa3[0][1]
    blocks("sa3l1a", W[:, :3], g)
    blocks("sa3l1b", W[:, 3:], g)
    blocks("sa3l2", npa(sa3[1][0]), sa3[1][1])
    blocks("sa3l3", npa(sa3[2][0]), sa3[2][1])

    W, g = npa(fp3[0][0]), fp3[0][1]
    blocks("fp3l1a", W[:, :256], g)
    blocks("fp3za", W[:, 256:], g)
    blocks("fp3l2", npa(fp3[1][0]), fp3[1][1])

    W, g = npa(fp2[0][0]), fp2[0][1]
    blocks("fp2l1a", W[:, :128], g)
    blocks("fp2za", W[:, 128:], g)
    blocks("fp2l2", npa(fp2[1][0]), fp2[1][1])

    blocks("fp1z", npa(fp1[0][0]), fp1[0][1])
    blocks("fp1l2", npa(fp1[1][0]), fp1[1][1])
    blocks("fp1l3", npa(fp1[2][0]), fp1[2][1])

    blocks("cls1", npa(cls1[0][0]), cls1[0][1])
    blocks("cls2", npa(params["cls2_w"]), np.ones((2,), np.float32))
    return out


def _wrap16(a):
    a = np.asarray(a, np.int16).reshape(-1).reshape(-1, 16).T  # (16, n/16)
    return np.ascontiguousarray(np.tile(a, (8, 1)))


# ----------------------------------------------------------------- device kernel

def _build_nc():
    nc = bacc.Bacc()
    din = {}

    def decl(name, shape, dt=F32):
        din[name] = nc.dram_tensor(name, list(shape), dt, kind="ExternalInput")

    decl("gx1", (3, 65536))
    decl("gx2", (3, 16384))
    decl("gx3", (3, 4096))
    decl("i2", (128, 1024), I16)
    decl("i3", (128, 256), I16)
    decl("f3i", (128, 96), I16)
    decl("f2i", (128, 384), I16)
    decl("f1i", (128, 1536), I16)
    decl("f3w", (1, 1536))
    decl("f2w", (1, 6144))
    decl("f1w", (1, 24576))
    for tag, (cin, cout) in WSPEC.items():
        for ki in range(_nblk(cin)):
            ksz = min(128, cin - ki * 128)
            for mi in range(_nblk(cout)):
                msz = min(128, cout - mi * 128)
                decl(f"{tag}_{ki}_{mi}", (ksz, msz))
    dout = nc.dram_tensor("out", [2, 8192], F32, kind="ExternalOutput")

    dma = nc.default_dma_engine

    with tile.TileContext(nc) as tc:
        with tc.tile_pool(name="wp", bufs=1) as wp, \
             tc.tile_pool(name="pp", bufs=6, space="PSUM") as pp, \
             tc.tile_pool(name="per", bufs=1) as per:

            # --- load weights/indices, build constants
            W = {}
            for tag, (cin, cout) in WSPEC.items():
                for ki in range(_nblk(cin)):
                    ksz = min(128, cin - ki * 128)
                    for mi in range(_nblk(cout)):
                        msz = min(128, cout - mi * 128)
                        nm = f"{tag}_{ki}_{mi}"
                        t = wp.tile([ksz, msz], F32, tag=nm, name="wt")
                        dma.dma_start(out=t[:], in_=din[nm][:])
                        W[(tag, ki, mi)] = t

            ident = wp.tile([128, 128], F32, tag="ident", name="ident")
            make_identity(nc, ident[:])
            ones1 = wp.tile([1, 128], F32, tag="ones1", name="ones1")
            nc.vector.memset(ones1[:], 1.0)

            def ldidx(nm, cols):
                t = wp.tile([128, cols], I16, tag=nm, name="idx")
                dma.dma_start(out=t[:], in_=din[nm][:])
                return t

            i2t = ldidx("i2", 1024)
            i3t = ldidx("i3", 256)
            f3it = ldidx("f3i", 96)
            f2it = ldidx("f2i", 384)
            f1it = ldidx("f1i", 1536)

            def lin(dst_sl, contribs, eng, add_src=None):
                # dst_sl/psum free size must be <= 512
                ps = pp.tile(list(dst_sl.shape), F32, tag="ps", name="ps")
                last = len(contribs) - (0 if add_src is not None else 1)
                for i, (w, r) in enumerate(contribs):
                    nc.tensor.matmul(ps[:], w[:], r,
                                     start=(i == 0), stop=(i == last))
                if add_src is not None:
                    nc.tensor.matmul(ps[:], ident[:], add_src,
                                     start=False, stop=True)
                if eng == "s":
                    nc.scalar.activation(dst_sl, ps[:], RELU)
                elif eng == "v":
                    nc.vector.tensor_scalar_max(dst_sl, ps[:], 0.0)
                else:
                    nc.vector.tensor_copy(out=dst_sl, in_=ps[:])

            def wbcast(dst3, wsrc_sl, n3):
                # dst3: (128, n3//3, 3) tile <- broadcast of wsrc_sl (1, n3)
                for c in range(0, n3, 384):
                    psb = pp.tile([128, 128, 3], F32, tag="psb", name="psb",
                                  bufs=2)
                    nc.tensor.matmul(psb[:], ones1[:], wsrc_sl[:, c:c + 384],
                                     start=True, stop=True)
                    nc.vector.tensor_copy(out=dst3[:, c // 3:c // 3 + 128, :],
                                          in_=psb[:])

            # =================== SA1 ===================
            l1p = per.tile([128, 2048], F32, tag="l1p", name="l1p")
            with tc.tile_pool(name="sa1", bufs=1) as sp:
                for t in range(8):  # chunks of 8192 cols (256 groups)
                    X = sp.tile([3, 8192], F32, tag="X", name="X")
                    dma.dma_start(out=X[:], in_=din["gx1"][:, t * 8192:(t + 1) * 8192])
                    A1 = sp.tile([64, 8192], F32, tag="A1", name="A1")
                    for n in range(0, 8192, 512):
                        lin(A1[:, n:n + 512],
                            [(W[("sa1l1", 0, 0)], X[:, n:n + 512])], "s")
                    A2 = sp.tile([64, 8192], F32, tag="A2", name="A2")
                    for n in range(0, 8192, 512):
                        lin(A2[:, n:n + 512],
                            [(W[("sa1l2", 0, 0)], A1[:, n:n + 512])], "v")
                    A3 = sp.tile([128, 256, 32], F32, tag="A3", name="A3")
                    for n in range(0, 8192, 512):
                        g = n // 32
                        lin(A3[:, g:g + 16, :],
                            [(W[("sa1l3", 0, 0)], A2[:, n:n + 512])], "s")
                    nc.vector.tensor_reduce(out=l1p[:, t * 256:(t + 1) * 256],
                                            in_=A3[:], axis=AX_X, op=OP_MAX)

            # =================== SA2 ===================
            l2pa = per.tile([128, 512], F32, tag="l2pa", name="l2pa")
            l2pb = per.tile([128, 512], F32, tag="l2pb", name="l2pb")
            with tc.tile_pool(name="sa2", bufs=1) as sp:
                for t in range(4):  # chunks of 4096 cols (128 groups)
                    G2 = sp.tile([128, 4096], F32, tag="G2", name="G2")
                    nc.gpsimd.ap_gather(out_ap=G2[:], in_ap=l1p[:],
                                        idxs_ap=i2t[:, t * 256:(t + 1) * 256],
                                        channels=128, num_elems=2048, d=1,
                                        num_idxs=4096)
                    X2 = sp.tile([3, 4096], F32, tag="X2", name="X2")
                    dma.dma_start(out=X2[:], in_=din["gx2"][:, t * 4096:(t + 1) * 4096])
                    B1 = sp.tile([128, 4096], F32, tag="B1", name="B1")
                    for n in range(0, 4096, 512):
                        lin(B1[:, n:n + 512],
                            [(W[("sa2l1a", 0, 0)], X2[:, n:n + 512]),
                             (W[("sa2l1b", 0, 0)], G2[:, n:n + 512])], "s")
                    B2 = sp.tile([128, 4096], F32, tag="B2", name="B2")
                    for n in range(0, 4096, 512):
                        lin(B2[:, n:n + 512],
                            [(W[("sa2l2", 0, 0)], B1[:, n:n + 512])], "v")
                    B3a = sp.tile([128, 128, 32], F32, tag="B3a", name="B3a")
                    B3b = sp.tile([128, 128, 32], F32, tag="B3b", name="B3b")
                    for n in range(0, 4096, 512):
                        g = n // 32
                        lin(B3a[:, g:g + 16, :],
                            [(W[("sa2l3", 0, 0)], B2[:, n:n + 512])], "s")
                        lin(B3b[:, g:g + 16, :],
                            [(W[("sa2l3", 0, 1)], B2[:, n:n + 512])], "v")
                    nc.vector.tensor_reduce(out=l2pa[:, t * 128:(t + 1) * 128],
                                            in_=B3a[:], axis=AX_X, op=OP_MAX)
                    nc.vector.tensor_reduce(out=l2pb[:, t * 128:(t + 1) * 128],
                                            in_=B3b[:], axis=AX_X, op=OP_MAX)

            # =================== SA3 ===================
            l3p = [per.tile([128, 128], F32, tag=f"l3p{i}", name="l3p")
                   for i in range(4)]
            with tc.tile_pool(name="sa3", bufs=1) as sp:
                for t in range(2):  # chunks of 2048 cols (64 groups)
                    G3a = sp.tile([128, 2048], F32, tag="G3a", name="G3a")
                    G3b = sp.tile([128, 2048], F32, tag="G3b", name="G3b")
                    idx_sl = i3t[:, t * 128:(t + 1) * 128]
                    nc.gpsimd.ap_gather(out_ap=G3a[:], in_ap=l2pa[:],
                                        idxs_ap=idx_sl, channels=128,
                                        num_elems=512, d=1, num_idxs=2048)
                    nc.gpsimd.ap_gather(out_ap=G3b[:], in_ap=l2pb[:],
                                        idxs_ap=idx_sl, channels=128,
                                        num_elems=512, d=1, num_idxs=2048)
                    X3 = sp.tile([3, 2048], F32, tag="X3", name="X3")
                    dma.dma_start(out=X3[:], in_=din["gx3"][:, t * 2048:(t + 1) * 2048])
                    C1 = [sp.tile([128, 2048], F32, tag=f"C1{m}", name="C1")
                          for m in range(2)]
                    for m in range(2):
                        for n in range(0, 2048, 512):
                            lin(C1[m][:, n:n + 512],
                                [(W[("sa3l1a", 0, m)], X3[:, n:n + 512]),
                                 (W[("sa3l1b", 0, m)], G3a[:, n:n + 512]),
                                 (W[("sa3l1b", 1, m)], G3b[:, n:n + 512])],
                                "s" if m == 0 else "v")
                    C2 = [sp.tile([128, 2048], F32, tag=f"C2{m}", name="C2")
                          for m in range(2)]
                    for m in range(2):
                        for n in range(0, 2048, 512):
                            lin(C2[m][:, n:n + 512],
                                [(W[("sa3l2", 0, m)], C1[0][:, n:n + 512]),
                                 (W[("sa3l2", 1, m)], C1[1][:, n:n + 512])],
                                "s" if m == 1 else "v")
                    C3 = [sp.tile([128, 64, 32], F32, tag=f"C3{m}", name="C3")
                          for m in range(4)]
                    for m in range(4):
                        for n in range(0, 2048, 512):
                            g = n // 32
                            lin(C3[m][:, g:g + 16, :],
                                [(W[("sa3l3", 0, m)], C2[0][:, n:n + 512]),
                                 (W[("sa3l3", 1, m)], C2[1][:, n:n + 512])],
                                "s" if m % 2 == 0 else "v")
                    for m in range(4):
                        nc.vector.tensor_reduce(out=l3p[m][:, t * 64:(t + 1) * 64],
                                                in_=C3[m][:], axis=AX_X, op=OP_MAX)

            # =================== FP3 ===================
            l2fa = per.tile([128, 512], F32, tag="l2fa", name="l2fa")
            l2fb = per.tile([128, 512], F32, tag="l2fb", name="l2fb")
            with tc.tile_pool(name="fp3", bufs=1) as sp:
                Z3 = [sp.tile([128, 128], F32, tag=f"Z3{m}", name="Z3")
                      for m in range(2)]
                for m in range(2):
                    lin(Z3[m][:],
                        [(W[("fp3za", k, m)], l3p[k][:]) for k in range(4)],
                        None)
                f3wt = sp.tile([1, 1536], F32, tag="f3wt", name="f3wt")
                dma.dma_start(out=f3wt[:], in_=din["f3w"][:])
                WB3 = sp.tile([128, 512, 3], F32, tag="WB3", name="WB3")
                wbcast(WB3, f3wt, 1536)
                WS3 = [sp.tile([128, 512], F32, tag=f"WS3{m}", name="WS3")
                       for m in range(2)]
                for m in range(2):
                    G3f = sp.tile([128, 512, 3], F32, tag="G3f", name="G3f",
                                  bufs=2)
                    nc.gpsimd.ap_gather(out_ap=G3f[:], in_ap=Z3[m][:],
                                        idxs_ap=f3it[:], channels=128,
                                        num_elems=128, d=1, num_idxs=1536)
                    Pm3 = sp.tile([128, 512, 3], F32, tag="Pm3", name="Pm3",
                                  bufs=2)
                    nc.vector.tensor_tensor(out=Pm3[:], in0=G3f[:], in1=WB3[:],
                                            op=OP_MULT)
                    nc.vector.tensor_reduce(out=WS3[m][:], in_=Pm3[:],
                                            axis=AX_X, op=OP_ADD)
                D1 = [sp.tile([128, 512], F32, tag=f"D1{m}", name="D1")
                      for m in range(2)]
                for m in range(2):
                    lin(D1[m][:],
                        [(W[("fp3l1a", 0, m)], l2pa[:]),
                         (W[("fp3l1a", 1, m)], l2pb[:])],
                        "s" if m == 0 else "v", add_src=WS3[m][:])
                for m, dst in enumerate((l2fa, l2fb)):
                    lin(dst[:],
                        [(W[("fp3l2", 0, m)], D1[0][:]),
                         (W[("fp3l2", 1, m)], D1[1][:])],
                        "s" if m == 1 else "v")

            # =================== FP2 ===================
            l1f = per.tile([128, 2048], F32, tag="l1f", name="l1f")
            with tc.tile_pool(name="fp2", bufs=1) as sp:
                Z2 = [sp.tile([128, 512], F32, tag=f"Z2{m}", name="Z2")
                      for m in range(2)]
                for m in range(2):
                    lin(Z2[m][:],
                        [(W[("fp2za", 0, m)], l2fa[:]),
                         (W[("fp2za", 1, m)], l2fb[:])], None)
                f2wt = sp.tile([1, 6144], F32, tag="f2wt", name="f2wt")
                dma.dma_start(out=f2wt[:], in_=din["f2w"][:])
                WB2 = sp.tile([128, 2048, 3], F32, tag="WB2", name="WB2")
                wbcast(WB2, f2wt, 6144)
                WS2 = [sp.tile([128, 2048], F32, tag=f"WS2{m}", name="WS2")
                       for m in range(2)]
                for m in range(2):
                    G2f = sp.tile([128, 2048, 3], F32, tag="G2f", name="G2f")
                    nc.gpsimd.ap_gather(out_ap=G2f[:], in_ap=Z2[m][:],
                                        idxs_ap=f2it[:], channels=128,
                                        num_elems=512, d=1, num_idxs=6144)
                    Pm2 = sp.tile([128, 2048, 3], F32, tag="Pm2", name="Pm2")
                    nc.vector.tensor_tensor(out=Pm2[:], in0=G2f[:], in1=WB2[:],
                                            op=OP_MULT)
                    nc.vector.tensor_reduce(out=WS2[m][:], in_=Pm2[:],
                                            axis=AX_X, op=OP_ADD)
                E1 = [sp.tile([128, 2048], F32, tag=f"E1{m}", name="E1")
                      for m in range(2)]
                for m in range(2):
                    for n in range(0, 2048, 512):
                        lin(E1[m][:, n:n + 512],
                            [(W[("fp2l1a", 0, m)], l1p[:, n:n + 512])],
                            "s" if m == 0 else "v",
                            add_src=WS2[m][:, n:n + 512])
                for n in range(0, 2048, 512):
                    lin(l1f[:, n:n + 512],
                        [(W[("fp2l2", 0, 0)], E1[0][:, n:n + 512]),
                         (W[("fp2l2", 1, 0)], E1[1][:, n:n + 512])], "s")

            # =================== FP1 + classifier ===================
            with tc.tile_pool(name="fp1", bufs=1) as sp:
                Z1 = sp.tile([128, 2048], F32, tag="Z1", name="Z1")
                for n in range(0, 2048, 512):
                    lin(Z1[:, n:n + 512],
                        [(W[("fp1z", 0, 0)], l1f[:, n:n + 512])], None)
                F1 = sp.tile([128, 8192], F32, tag="big", name="F1", bufs=2)
                for t in range(8):  # 1024 points / 3072 idxs per chunk
                    f1wt = sp.tile([1, 3072], F32, tag="f1wt", name="f1wt")
                    dma.dma_start(out=f1wt[:],
                                  in_=din["f1w"][:, t * 3072:(t + 1) * 3072])
                    WB1 = sp.tile([128, 1024, 3], F32, tag="WB1", name="WB1")
                    wbcast(WB1, f1wt, 3072)
                    G1 = sp.tile([128, 1024, 3], F32, tag="G1", name="G1")
                    nc.gpsimd.ap_gather(out_ap=G1[:], in_ap=Z1[:],
                                        idxs_ap=f1it[:, t * 192:(t + 1) * 192],
                                        channels=128, num_elems=2048, d=1,
                                        num_idxs=3072)
                    Pm1 = sp.tile([128, 1024, 3], F32, tag="Pm1", name="Pm1")
                    nc.vector.tensor_tensor(out=Pm1[:], in0=G1[:], in1=WB1[:],
                                            op=OP_MULT)
                    R1 = sp.tile([128, 1024], F32, tag="R1", name="R1")
                    nc.vector.tensor_reduce(out=R1[:], in_=Pm1[:], axis=AX_X,
                                            op=OP_ADD)
                    nc.scalar.activation(F1[:, t * 1024:(t + 1) * 1024],
                                         R1[:], RELU)
                T2 = sp.tile([128, 8192], F32, tag="big", name="T2", bufs=2)
                for n in range(0, 8192, 512):
                    lin(T2[:, n:n + 512],
                        [(W[("fp1l2", 0, 0)], F1[:, n:n + 512])], "v")
                l0f = sp.tile([128, 8192], F32, tag="big", name="l0f", bufs=2)
                for n in range(0, 8192, 512):
                    lin(l0f[:, n:n + 512],
                        [(W[("fp1l3", 0, 0)], T2[:, n:n + 512])], "s")
                H = sp.tile([128, 8192], F32, tag="big", name="H", bufs=2)
                for n in range(0, 8192, 512):
                    lin(H[:, n:n + 512],
                        [(W[("cls1", 0, 0)], l0f[:, n:n + 512])], "v")
                for n in range(0, 8192, 512):
                    OUTt = sp.tile([2, 512], F32, tag="OUTt", name="OUTt",
                                   bufs=2)
                    lin(OUTt[:], [(W[("cls2", 0, 0)], H[:, n:n + 512])], None)
                    dma.dma_start(out=dout[:, n:n + 512], in_=OUTt[:])

    nc.compile()
    return nc


_NC = None


def _get_nc():
    global _NC
    if _NC is None:
        _NC = _build_nc()
    return _NC


def kernel(xyz, params):
    global LAST_RESULTS
    xyz = np.asarray(xyz, np.float32)
    hd = _host_indices(xyz)
    wmap = _prep_weights(params)

    in_maps = []
    for b in range(8):
        m = dict(wmap)
        m["gx1"] = np.ascontiguousarray(
            hd["gx1"][b].reshape(65536, 3).T.astype(np.float32))
        m["gx2"] = np.ascontiguousarray(
            hd["gx2"][b].reshape(16384, 3).T.astype(np.float32))
        m["gx3"] = np.ascontiguousarray(
            hd["gx3"][b].reshape(4096, 3).T.astype(np.float32))
        m["i2"] = _wrap16(hd["idx2"][b])
        m["i3"] = _wrap16(hd["idx3"][b])
        m["f3i"] = _wrap16(hd["i3f"][b])
        m["f2i"] = _wrap16(hd["i2f"][b])
        m["f1i"] = _wrap16(hd["i1f"][b])
        m["f3w"] = np.ascontiguousarray(hd["w3"][b].reshape(1, -1).astype(np.float32))
        m["f2w"] = np.ascontiguousarray(hd["w2"][b].reshape(1, -1).astype(np.float32))
        m["f1w"] = np.ascontiguousarray(hd["w1"][b].reshape(1, -1).astype(np.float32))
        in_maps.append(m)

    nc = _get_nc()
    global LAST_EXEC_S
    import time as _time
    _t0 = _time.time()
    res = bass_utils.run_bass_kernel_spmd(nc, in_maps, list(range(8)),
                                          trace=TRACE)
    LAST_EXEC_S = _time.time() - _t0
    LAST_RESULTS = res
    return np.stack([np.asarray(res.results[b]["out"]) for b in range(8)])
